# revision 1
# baseline (speedup 1.0000x reference)
"""Causal attention (B=8, N=4096, D=64) on 8 trn2 NeuronCores.

Sharding: batch b -> core b (data parallel, no cross-core comms).

Per-core kernel (flash-attention style, fully transposed dataflow -- no
on-chip transposes anywhere):
  inputs (host pre-layouts, fp16):
    qk    [64, nqb, 2, 512]  packed (kT | qT) chunks, d on partitions
    v_aug [128, N/128, 65]   k-tiled; col 64 = 1.0; padding-masked rows = 0
    cmasks[128, 4, 512]      causal 0/1 tiles per diagonal alignment
  for each q-block (512 wide), each causal k-tile PAIR (2 x 128):
    logitsT[k, q] = matmul(lhsT=kT_t [64,128], rhs=qT_blk [64,512])  (PSUM)
    expT = exp(logitsT_pair / sqrt(d))   one ACT op over [128,1024] -> SBUF
    diagonal halves: expT *= causal 0/1 mask                         (DVE)
    outT[d,q] (+)= matmul(lhsT=v_aug [128,65], rhs=expT [128,512])   (PSUM)
      -- v_aug col 64 is 1.0 => outT row 64 = the softmax denominators
  The MM2s are emitted one pair behind the MM1s/exp so the PE stream is
  [.. MM1s(p) MM2s(p-1) ..] and the exp latency is always hidden.
  per q-block epilogue (off the PE): r = 1/sums (DVE); broadcast r across
  partitions via a DRAM round-trip (partition-step-0 reads are DRAM-only);
  out = outT[0:64] * r (DVE); DMA to outT_dram[:, q-block]. The last
  q-block instead uses a PE outer-product broadcast (shorter tail chain).
  Host transposes outT_dram [64, N] back to [N, 64] at gather time.

Padding mask: host zeroes masked k rows of v_aug (incl. the ones column),
so masked keys contribute nothing to numerator or denominator -- exactly
equivalent to -inf logits.

Matmul operands are fp16 (1 cycle/row on the PE; fp32 PSUM accumulation);
measured rel err vs the fp32 reference is ~4e-4. Measured ~119.5 us/core
on trn2 (ACT-engine softmax-exp bound; PE co-saturated).
"""

import os
from contextlib import ExitStack

import numpy as np

B, N, D = 8, 4096, 64
QBLK = 512
KTILE = 128

LAST_RESULTS = None
_NC_CACHE = {}


def build(n=N, d=D, qblk=QBLK, ktile=KTILE, lg_bufs=3, acc_bufs=2, pb_bufs=6,
          op_dt="float16", epi_depth=1):
    import concourse.bass as bass
    import concourse.mybir as mybir
    import concourse.tile as tile
    from concourse import bacc

    f32 = mybir.dt.float32
    f32r = mybir.dt.float32r
    opd = getattr(mybir.dt, op_dt)   # matmul operand dtype (fp16 or f32r)
    qblk = min(qblk, n)
    nt = n // ktile          # number of k-tiles
    nqb = n // qblk          # number of q-blocks
    tpq = qblk // ktile      # k-tiles per q-block (diagonal span)

    nc = bacc.Bacc("TRN2", target_bir_lowering=False, debug=False,
                   enable_asserts=False)

    qk_d = nc.dram_tensor("qk", (d, nqb, 2, qblk), opd,
                          kind="ExternalInput").ap()
    v_d = nc.dram_tensor("v_aug", (128, nt, d + 1), opd,
                         kind="ExternalInput").ap()
    mk_d = nc.dram_tensor("cmasks", (128, tpq, qblk), opd,
                          kind="ExternalInput").ap()
    oT_d = nc.dram_tensor("outT", (d, n), f32, kind="ExternalOutput").ap()
    rs_d = nc.dram_tensor("rs_scratch", (nqb, qblk), f32,
                          kind="Internal").ap()

    scale = 1.0 / float(np.sqrt(d))

    with tile.TileContext(nc) as tc:
        with ExitStack() as ctx:
            singles = ctx.enter_context(tc.tile_pool(name="singles", bufs=1))
            pb_pool = ctx.enter_context(tc.tile_pool(name="pb", bufs=pb_bufs))
            small = ctx.enter_context(tc.tile_pool(name="small", bufs=2))
            ob_pool = ctx.enter_context(tc.tile_pool(name="ob", bufs=3))
            lg_pool = ctx.enter_context(
                tc.tile_pool(name="lg", bufs=lg_bufs, space="PSUM"))
            acc_pool = ctx.enter_context(
                tc.tile_pool(name="acc", bufs=acc_bufs, space="PSUM"))

            # --- resident inputs -------------------------------------------
            qk_sb = singles.tile([d, nqb, 2, qblk], opd)
            v_sb = singles.tile([128, nt, d + 1], opd)
            mk_sb = singles.tile([128, tpq, qblk], opd)

            # few big DMAs (per-DMA issue on the sync queue is ~650ns,
            # serial), first chunk first so compute starts early
            gchunk = max(1, nqb // 4)
            groups = [(c, min(nqb, c + gchunk)) for c in range(0, nqb, gchunk)]
            for gi, (c, ce) in enumerate(groups):
                nc.sync.dma_start(out=qk_sb[:, c:ce, :, :],
                                  in_=qk_d[:, c:ce, :, :])
                if gi == 0:
                    nc.sync.dma_start(out=mk_sb, in_=mk_d)
                vs, ve = c * tpq, min(nt, ce * tpq)
                nc.sync.dma_start(out=v_sb[:, vs:ve, :], in_=v_d[:, vs:ve, :])

            def kT_ap(t):
                c, r = divmod(t, tpq)
                return qk_sb[:, c, 0, r * ktile:(r + 1) * ktile]

            # --- main loop -------------------------------------------------
            def epilogue(acc, qs, qb, last=False):
                # normalize: out = outT[0:64] / sums (sums = row d of acc).
                # Steady state: DVE reciprocal + DRAM-round-trip partition
                # broadcast (all off the PE, which is busy).
                # Last q-block (nothing left to overlap): ACT reciprocal
                # exp(-ln(s)) + PE outer-product broadcast -- a much shorter
                # serial chain.
                if not last:
                    rsum = small.tile([1, qblk], f32, name="rsum")
                    nc.vector.reciprocal(rsum, acc[d:d + 1, :])
                    nc.sync.dma_start(out=rs_d[qb:qb + 1, :], in_=rsum)
                    rb = ob_pool.tile([d, qblk], f32, name="rb")
                    rs_slice = rs_d[qb:qb + 1, :]
                    brd = bass.AP(tensor=rs_slice.tensor,
                                  offset=rs_slice.offset,
                                  ap=[[0, d], list(rs_slice.ap[-1])])
                    nc.sync.dma_start(out=rb, in_=brd)
                    ob = ob_pool.tile([d, qblk], f32, name="ob")
                    nc.vector.tensor_mul(ob, acc[0:d, :], rb)
                    nc.sync.dma_start(out=oT_d[:, qs:qs + qblk], in_=ob)
                    return
                rsum = small.tile([1, qblk], f32, name="rsum")
                nc.vector.reciprocal(rsum, acc[d:d + 1, :])
                ones_f32 = small.tile([1, d], f32, name="ones_f32")
                nc.scalar.activation(
                    ones_f32, mk_sb[0:1, 0, 0:d],
                    mybir.ActivationFunctionType.Copy)
                bc = lg_pool.tile([d, qblk], f32, name="bc", tag="lg")
                nc.tensor.matmul(bc, lhsT=ones_f32, rhs=rsum,
                                 start=True, stop=True)
                bc_sb = ob_pool.tile([d, qblk], f32, name="rb")
                nc.vector.tensor_copy(bc_sb, bc)
                ob = ob_pool.tile([d, qblk], f32, name="ob")
                nc.vector.tensor_mul(ob, acc[0:d, :], bc_sb)
                nc.sync.dma_start(out=oT_d[:, qs:qs + qblk], in_=ob)

            # Per k-tile PAIR: emit MM1s + exp(s) [+ boundary masks], then
            # the deferred MM2s of the previous pair, so the PE stream
            # interleaves [... MM1s(p) MM2s(p-1) ...] and fills the exp
            # latency. Diagonal tiles (j = t - tpq*qb >= 0) read/write only
            # their live columns [128*j, qblk) in MM2 (and in MM1/exp when
            # that is free); the 128-wide boundary strip gets the
            # triangular 0/1 mask.
            mm2_q = []   # deferred MM2s: (acc, pb, t0, lows, qb, tlast)

            def flush_mm2():
                acc_, pb_, t0_, qb_, tlast_ = mm2_q.pop(0)
                for h in range(2):
                    t = t0_ + h
                    nc.tensor.matmul(
                        acc_,
                        lhsT=v_sb[:, t, :],
                        rhs=pb_[:, h, :],
                        start=(t == 0), stop=(t == tlast_),
                    )
                if t0_ + 1 == tlast_:   # last pair: normalize this q-block
                    epilogue(acc_, qb_ * qblk, qb_, last=(qb_ == nqb - 1))

            for qb in range(nqb):
                q_sl = qk_sb[:, qb, 1, :]
                acc = acc_pool.tile([d + 1, qblk], f32, name="acc", tag="acc")
                npairs = (tpq * qb + tpq) // 2
                tlast = 2 * npairs - 1
                for p in range(npairs):
                    t0 = 2 * p
                    lg = lg_pool.tile([128, 2, qblk], f32, name="lg")
                    pb = pb_pool.tile([128, 2, qblk], opd, name="pb")
                    for h in range(2):
                        nc.tensor.matmul(
                            lg[:, h, :],
                            lhsT=kT_ap(t0 + h),
                            rhs=q_sl,
                            start=True, stop=True,
                        )
                    nc.scalar.activation(
                        pb, lg, mybir.ActivationFunctionType.Exp,
                        scale=scale)
                    for h in range(2):
                        j = t0 + h - tpq * qb
                        if j >= 0:
                            nc.vector.tensor_mul(
                                pb[:, h, :], pb[:, h, :], mk_sb[:, j, :])
                    mm2_q.append((acc, pb, t0, qb, tlast))
                    if len(mm2_q) >= 2:
                        flush_mm2()
            while mm2_q:
                flush_mm2()

    nc.compile()
    return nc


def _get_nc(key="main", **kw):
    if key not in _NC_CACHE:
        _NC_CACHE[key] = build(**kw)
    return _NC_CACHE[key]


def _prep_core_inputs(q, k, v, attn_mask, b, n=N, d=D, ktile=KTILE,
                      qblk=QBLK, op_dt="float16"):
    npdt = np.float16 if op_dt == "float16" else np.float32
    qblk = min(qblk, n)
    nt = n // ktile
    nqb = n // qblk
    qT = q[b].T.astype(npdt)          # [d, n]
    kT = k[b].T.astype(npdt)
    qk = np.empty((d, nqb, 2, qblk), dtype=npdt)
    qk[:, :, 0, :] = kT.reshape(d, nqb, qblk)
    qk[:, :, 1, :] = qT.reshape(d, nqb, qblk)
    v_aug = np.ones((n, d + 1), dtype=np.float32)
    v_aug[:, :d] = v[b]
    v_aug *= (attn_mask[b] != 0).astype(np.float32)[:, None]
    v_aug = np.ascontiguousarray(
        v_aug.reshape(nt, ktile, d + 1).transpose(1, 0, 2)).astype(npdt)
    tpq = qblk // ktile
    # causal 0/1 mask per diagonal alignment j: keep where q >= k + 128*j
    y = np.arange(qblk)[None, None, :]
    x = np.arange(ktile)[:, None, None]
    jj = np.arange(tpq)[None, :, None]
    cmasks = (y - x - ktile * jj >= 0).astype(npdt)
    return {"qk": qk, "v_aug": v_aug, "cmasks": cmasks}


def kernel(q, k, v, attn_mask):
    global LAST_RESULTS
    q = np.asarray(q, dtype=np.float32)
    k = np.asarray(k, dtype=np.float32)
    v = np.asarray(v, dtype=np.float32)
    attn_mask = np.asarray(attn_mask)

    from concourse.bass_utils import run_bass_kernel_spmd

    nc = _get_nc()
    in_maps = [_prep_core_inputs(q, k, v, attn_mask, b) for b in range(B)]
    trace = bool(os.environ.get("BASS_TRACE"))
    last_err = None
    for attempt in range(3):
        try:
            LAST_RESULTS = run_bass_kernel_spmd(
                nc, in_maps, core_ids=list(range(B)), trace=trace)
            break
        except Exception as e:  # transient device-unrecoverable states clear
            last_err = e        # on the next execution attempt
            if "UNAVAILABLE" not in str(e) and "unrecoverable" not in str(e):
                raise
            import time as _time

            _time.sleep(2.0)
    else:
        raise last_err

    out = np.empty((B, N, D), dtype=np.float32)
    for b in range(B):
        out[b] = LAST_RESULTS.results[b]["outT"].T
    return out



# revision 6
# speedup vs baseline: 1.2961x; 1.2961x over previous
"""Causal attention (B=8, N=4096, D=64) on 8 trn2 NeuronCores.

Sharding: batch b -> core b (data parallel, no cross-core comms).

Per-core kernel (flash-attention style, fully transposed dataflow -- no
on-chip transposes anywhere):
  inputs (host pre-layouts, fp16):
    kpair [128, 16, 128]  kT tile pairs: partitions 0-63 = kT of even
                          tiles, 64-127 = kT of odd tiles (d on partitions)
    qq    [128, 8, 512]   qT blocks, duplicated on both partition halves
    v_aug [128, 32, 65]   k-tiled; col 64 = 1.0; padding-masked rows = 0
    tri   [128, 128]      lower-triangular 0/1 (keep where qcol >= krow)
  for each q-block (512 wide, processed LARGEST FIRST to spend the most
  PE cycles inside the HAM warm window), each causal k-tile PAIR:
    MM1 x2 ROW-TILED: logitsT[k,q] = matmul(kT_t [64,128], qT [64,512])
      The two K=64 matmuls of a pair sit at partition bases 0 and 64 ->
      tile_position (0,0)/(64,0) -> they run CONCURRENTLY in different
      row strips of the PE array (~518 cyc/pair instead of 1024).
    expT = exp(logitsT_pair / sqrt(d))  one ACT op over [128,1024] -> SBUF
    diagonal tiles: pb strip [128,128] *= tri                       (DVE)
    MM2 x2: outT[d,q] (+)= matmul(v_aug [128,65], expT [128,cs:512]) PSUM
      -- v_aug col 64 is 1.0 => outT row 64 = the softmax denominators
      -- diagonal tiles only touch their live columns [128j, 512): saves
         ~6k PE cycles/block and keeps the tri mask to one 128-wide strip
  The MM2s are emitted one pair behind the MM1s/exp so the PE stream is
  [.. MM1s(p) MM2s(p-1) ..] and the exp latency is hidden.
  per q-block epilogue (all off the PE, DMAs on the idle GpSimd queue):
  DMA sums row -> DRAM; DMA it back broadcast across 64 partitions
  (partition-step-0 reads are DRAM-only); out = acc[0:64] / sums via a
  single DVE tensor_tensor divide; DMA to outT_dram[:, q-block].
  Host transposes outT_dram [64, N] back to [N, 64] at gather time.

Padding mask: host zeroes masked k rows of v_aug (incl. the ones column),
so masked keys contribute nothing to numerator or denominator -- exactly
equivalent to -inf logits.

Matmul operands are fp16 (1 cycle/row on the PE; fp32 PSUM accumulation);
measured rel err vs the fp32 reference is ~4e-4.
"""

import os
from contextlib import ExitStack

import numpy as np

B, N, D = 8, 4096, 64
QBLK = 512
KTILE = 128

LAST_RESULTS = None
_NC_CACHE = {}


def build(n=N, d=D, qblk=QBLK, ktile=KTILE, lg_bufs=3, acc_bufs=2, pb_bufs=6,
          mm2_defer=1):
    import concourse.bass as bass
    import concourse.mybir as mybir
    import concourse.tile as tile
    from concourse import bacc

    f32 = mybir.dt.float32
    f16 = mybir.dt.float16
    nt = n // ktile          # number of k-tiles (32)
    nqb = n // qblk          # number of q-blocks (8)
    tpq = qblk // ktile      # k-tiles per q-block (diagonal span, 4)
    npr = nt // 2            # k-tile pairs total (16)

    nc = bacc.Bacc("TRN2", target_bir_lowering=False, debug=False,
                   enable_asserts=False)

    kp_d = nc.dram_tensor("kpair", (2 * d, npr, ktile), f16,
                          kind="ExternalInput").ap()
    qq_d = nc.dram_tensor("qq", (2 * d, nqb, qblk), f16,
                          kind="ExternalInput").ap()
    v_d = nc.dram_tensor("v_aug", (ktile, nt, d + 1), f16,
                         kind="ExternalInput").ap()
    tri_d = nc.dram_tensor("tri", (ktile, ktile), f16,
                           kind="ExternalInput").ap()
    oT_d = nc.dram_tensor("outT", (d, n), f32, kind="ExternalOutput").ap()
    rs_d = nc.dram_tensor("rs_scratch", (nqb, qblk), f32,
                          kind="Internal").ap()

    scale = 1.0 / float(np.sqrt(d))

    with tile.TileContext(nc) as tc:
        with ExitStack() as ctx:
            singles = ctx.enter_context(tc.tile_pool(name="singles", bufs=1))
            pb_pool = ctx.enter_context(tc.tile_pool(name="pb", bufs=pb_bufs))
            ob_pool = ctx.enter_context(tc.tile_pool(name="ob", bufs=4))
            lg_pool = ctx.enter_context(
                tc.tile_pool(name="lg", bufs=lg_bufs, space="PSUM"))
            acc_pool = ctx.enter_context(
                tc.tile_pool(name="acc", bufs=acc_bufs, space="PSUM"))

            # --- resident inputs -------------------------------------------
            kp_sb = singles.tile([2 * d, npr, ktile], f16)
            qq_sb = singles.tile([2 * d, nqb, qblk], f16)
            v_sb = singles.tile([ktile, nt, d + 1], f16)
            tri_sb = singles.tile([ktile, ktile], f16)

            # First q-block processed is the LAST one (qb = nqb-1); it needs
            # the whole of k progressively but only its own q block at once.
            # Small first chunks so compute starts early; per-DMA issue on
            # the sync queue is ~650ns, serial.
            nc.sync.dma_start(out=kp_sb[:, 0:4, :], in_=kp_d[:, 0:4, :])
            nc.sync.dma_start(out=qq_sb[:, nqb - 1, :],
                              in_=qq_d[:, nqb - 1, :])
            nc.sync.dma_start(out=tri_sb, in_=tri_d)
            nc.sync.dma_start(out=v_sb[:, 0:8, :], in_=v_d[:, 0:8, :])
            nc.sync.dma_start(out=kp_sb[:, 4:npr, :], in_=kp_d[:, 4:npr, :])
            nc.sync.dma_start(out=v_sb[:, 8:nt, :], in_=v_d[:, 8:nt, :])
            nc.sync.dma_start(out=qq_sb[:, 4:nqb - 1, :],
                              in_=qq_d[:, 4:nqb - 1, :])
            nc.sync.dma_start(out=qq_sb[:, 0:4, :], in_=qq_d[:, 0:4, :])

            # --- main loop -------------------------------------------------
            def epilogue(acc, qb):
                # normalize: out = acc[0:64] * (1/sums) (sums = row d of acc).
                # reciprocal_approx_fast (1 custom-DVE op, ~18-bit, ~5x faster
                # than InstReciprocal) reads the PSUM row and writes SBUF; a
                # DRAM round-trip broadcasts it across the 64 partitions
                # (partition-step-0 reads are DRAM-only).
                qs = qb * qblk
                # custom-DVE ops misread PSUM operands on HW (measured ~10%
                # error) -- stage the sums row through SBUF first.
                ssum = ob_pool.tile([1, qblk], f32, name="ssum")
                nc.vector.tensor_copy(ssum, acc[d:d + 1, :])
                rsum = ob_pool.tile([1, qblk], f32, name="rsum")
                nc.vector.reciprocal_approx_fast(out=rsum, in_=ssum)
                nc.gpsimd.dma_start(out=rs_d[qb:qb + 1, :], in_=rsum)
                rb = ob_pool.tile([d, qblk], f32, name="rb")
                rs_slice = rs_d[qb:qb + 1, :]
                brd = bass.AP(tensor=rs_slice.tensor,
                              offset=rs_slice.offset,
                              ap=[[0, d], list(rs_slice.ap[-1])])
                nc.gpsimd.dma_start(out=rb, in_=brd)
                ob = ob_pool.tile([d, qblk], f32, name="ob")
                nc.vector.tensor_mul(ob, acc[0:d, :], rb)
                nc.gpsimd.dma_start(out=oT_d[:, qs:qs + qblk], in_=ob)

            # Per k-tile pair: emit the two ROW-TILED MM1s + exp(pair)
            # [+ boundary tri-mask], then the deferred MM2s of the previous
            # pair, so the PE stream interleaves [.. MM1s(p) MM2s(p-1) ..]
            # and fills the exp latency. Diagonal tiles (j = t - tpq*qb >= 0)
            # read/write only their live columns [128*j, qblk) in MM2.
            mm2_q = []   # deferred MM2s: (acc, pb, (t0, t1), qb, tlast)

            def flush_mm2():
                acc_, pb_, tiles_, qb_, tlast_ = mm2_q.pop(0)
                for h, t in enumerate(tiles_):
                    j = t - tpq * qb_
                    cs = ktile * j if j > 0 else 0
                    nc.tensor.matmul(
                        acc_[:, cs:],
                        lhsT=v_sb[:, t, :],
                        rhs=pb_[:, h, cs:],
                        start=(t == 0), stop=(t == tlast_),
                    )
                if tiles_[1] == tlast_:   # last pair: normalize this q-block
                    epilogue(acc_, qb_)

            for qb in reversed(range(nqb)):
                ntiles = tpq * (qb + 1)
                npairs = ntiles // 2
                tlast = ntiles - 1
                acc = acc_pool.tile([d + 1, qblk], f32, name="acc", tag="acc")
                for p in range(npairs):
                    t0, t1 = 2 * p, 2 * p + 1
                    lg = lg_pool.tile([128, 2, qblk], f32, name="lg")
                    nc.tensor.matmul(
                        lg[:, 0, :],
                        lhsT=kp_sb[0:d, p, :],
                        rhs=qq_sb[0:d, qb, :],
                        start=True, stop=True,
                    )
                    nc.tensor.matmul(
                        lg[:, 1, :],
                        lhsT=kp_sb[d:2 * d, p, :],
                        rhs=qq_sb[d:2 * d, qb, :],
                        start=True, stop=True,
                    )
                    pb = pb_pool.tile([128, 2, qblk], f16, name="pb")
                    nc.scalar.activation(
                        pb, lg, mybir.ActivationFunctionType.Exp,
                        scale=scale)
                    for h, t in ((0, t0), (1, t1)):
                        j = t - tpq * qb
                        if j >= 0:
                            nc.vector.tensor_mul(
                                pb[:, h, ktile * j:ktile * (j + 1)],
                                pb[:, h, ktile * j:ktile * (j + 1)],
                                tri_sb)
                    mm2_q.append((acc, pb, (t0, t1), qb, tlast))
                    if len(mm2_q) > mm2_defer:
                        flush_mm2()
            while mm2_q:
                flush_mm2()

    nc.compile()
    return nc


def _get_nc(key="main", **kw):
    if key not in _NC_CACHE:
        _NC_CACHE[key] = build(**kw)
    return _NC_CACHE[key]


def _prep_core_inputs(q, k, v, attn_mask, b, n=N, d=D, ktile=KTILE,
                      qblk=QBLK):
    nt = n // ktile
    nqb = n // qblk
    npr = nt // 2
    kT = k[b].T.astype(np.float16)    # [d, n]
    qT = q[b].T.astype(np.float16)
    # kpair[0:64, p, :] = kT tile 2p; kpair[64:128, p, :] = kT tile 2p+1
    kpair = np.ascontiguousarray(
        kT.reshape(d, npr, 2, ktile).transpose(2, 0, 1, 3)
    ).reshape(2 * d, npr, ktile)
    # qq: qT blocks duplicated on both partition halves
    qq = np.empty((2 * d, nqb, qblk), dtype=np.float16)
    qq[0:d] = qT.reshape(d, nqb, qblk)
    qq[d:2 * d] = qq[0:d]
    v_aug = np.ones((n, d + 1), dtype=np.float32)
    v_aug[:, :d] = v[b]
    v_aug *= (attn_mask[b] != 0).astype(np.float32)[:, None]
    v_aug = np.ascontiguousarray(
        v_aug.reshape(nt, ktile, d + 1).transpose(1, 0, 2)
    ).astype(np.float16)
    # tri[kk, qc] = 1 iff qc >= kk (keep)
    tri = (np.arange(ktile)[None, :] >= np.arange(ktile)[:, None]
           ).astype(np.float16)
    return {"kpair": kpair, "qq": qq, "v_aug": v_aug, "tri": tri}


def kernel(q, k, v, attn_mask):
    global LAST_RESULTS
    q = np.asarray(q, dtype=np.float32)
    k = np.asarray(k, dtype=np.float32)
    v = np.asarray(v, dtype=np.float32)
    attn_mask = np.asarray(attn_mask)

    from concourse.bass_utils import run_bass_kernel_spmd

    nc = _get_nc()
    in_maps = [_prep_core_inputs(q, k, v, attn_mask, b) for b in range(B)]
    trace = bool(os.environ.get("BASS_TRACE"))
    last_err = None
    for attempt in range(3):
        try:
            LAST_RESULTS = run_bass_kernel_spmd(
                nc, in_maps, core_ids=list(range(B)), trace=trace)
            break
        except Exception as e:  # transient device-unrecoverable states clear
            last_err = e        # on the next execution attempt
            if "UNAVAILABLE" not in str(e) and "unrecoverable" not in str(e):
                raise
            import time as _time

            _time.sleep(2.0)
    else:
        raise last_err

    out = np.empty((B, N, D), dtype=np.float32)
    for b in range(B):
        out[b] = LAST_RESULTS.results[b]["outT"].T
    return out


# revision 9
# speedup vs baseline: 1.5712x; 1.2122x over previous
"""Causal attention (B=8, N=4096, D=64) on 8 trn2 NeuronCores.

Sharding: batch b -> core b (data parallel, no cross-core comms).

Per-core kernel (flash-attention style, fully transposed dataflow -- no
on-chip transposes anywhere):
  inputs (host pre-layouts, fp16):
    kpair [128, 16, 128]  kT tile pairs: partitions 0-63 = kT of even
                          tiles, 64-127 = kT of odd tiles (d on partitions)
    qq    [128, 8, 512]   qT blocks, duplicated on both partition halves
    v_aug [128, 32, 65]   k-tiled; col 64 = 1.0; padding-masked rows = 0
    tri   [128, 128]      lower-triangular 0/1 (keep where qcol >= krow)
  for each q-block (512 wide, ascending), each causal k-tile PAIR:
    MM1 x2 ROW-TILED: logitsT[k,q] = matmul(kT_t [64,128], qT [64,512])
      The two K=64 matmuls of a pair sit at partition bases 0 and 64 ->
      tile_position (0,0)/(64,0) -> they run CONCURRENTLY in different
      row strips of the PE array (~518 cyc/pair instead of 1024).
    expT = exp(logitsT_pair / sqrt(d))  one ACT op over [128,1024] -> SBUF
    diagonal tiles: pb strip [128,128] *= tri                       (DVE)
    MM2 x2: outT[d,q] (+)= matmul(v_aug [128,65], expT [128,cs:512]) PSUM
      -- v_aug col 64 is 1.0 => outT row 64 = the softmax denominators
      -- diagonal tiles only touch their live columns [128j, 512): saves
         ~6k PE cycles/block and keeps the tri mask to one 128-wide strip
  The MM2s are emitted one pair behind the MM1s/exp so the PE stream is
  [.. MM1s(p) MM2s(p-1) ..] and the exp latency is hidden.
  per q-block epilogue (all off the PE, DMAs on the idle GpSimd queue):
  DMA sums row -> DRAM; DMA it back broadcast across 64 partitions
  (partition-step-0 reads are DRAM-only); out = acc[0:64] / sums via a
  single DVE tensor_tensor divide; DMA to outT_dram[:, q-block].
  Host transposes outT_dram [64, N] back to [N, 64] at gather time.

Padding mask: host zeroes masked k rows of v_aug (incl. the ones column),
so masked keys contribute nothing to numerator or denominator -- exactly
equivalent to -inf logits.

Matmul operands are fp16 (1 cycle/row on the PE; fp32 PSUM accumulation);
measured rel err vs the fp32 reference is ~4e-4.
"""

import os
from contextlib import ExitStack

import numpy as np

B, N, D = 8, 4096, 64
QBLK = 512
KTILE = 128

LAST_RESULTS = None
_NC_CACHE = {}


def build(n=N, d=D, qblk=QBLK, ktile=KTILE, lg_bufs=3, acc_bufs=2, pb_bufs=6,
          mm2_defer=1):
    import concourse.bass as bass
    import concourse.mybir as mybir
    import concourse.tile as tile
    from concourse import bacc

    f32 = mybir.dt.float32
    f16 = mybir.dt.float16
    nt = n // ktile          # number of k-tiles (32)
    nqb = n // qblk          # number of q-blocks (8)
    tpq = qblk // ktile      # k-tiles per q-block (diagonal span, 4)
    npr = nt // 2            # k-tile pairs total (16)

    nc = bacc.Bacc("TRN2", target_bir_lowering=False, debug=False,
                   enable_asserts=False)

    kp_d = nc.dram_tensor("kpair", (2 * d, npr, ktile), f16,
                          kind="ExternalInput").ap()
    qq_d = nc.dram_tensor("qq", (2 * d, nqb, qblk), f16,
                          kind="ExternalInput").ap()
    v_d = nc.dram_tensor("v_aug", (ktile, nt, d + 1), f16,
                         kind="ExternalInput").ap()
    tri_d = nc.dram_tensor("tri", (ktile, ktile), f16,
                           kind="ExternalInput").ap()
    oT_d = nc.dram_tensor("outT", (d, n), f32, kind="ExternalOutput").ap()
    rs_d = nc.dram_tensor("rs_scratch", (nqb, qblk), f32,
                          kind="Internal").ap()

    scale = 1.0 / float(np.sqrt(d))

    with tile.TileContext(nc) as tc:
        with ExitStack() as ctx:
            singles = ctx.enter_context(tc.tile_pool(name="singles", bufs=1))
            pb_pool = ctx.enter_context(tc.tile_pool(name="pb", bufs=pb_bufs))
            ob_pool = ctx.enter_context(tc.tile_pool(name="ob", bufs=4))
            lg_pool = ctx.enter_context(
                tc.tile_pool(name="lg", bufs=lg_bufs, space="PSUM"))
            acc_pool = ctx.enter_context(
                tc.tile_pool(name="acc", bufs=acc_bufs, space="PSUM"))

            # --- resident inputs -------------------------------------------
            kp_sb = singles.tile([2 * d, npr, ktile], f16)
            qq_sb = singles.tile([2 * d, nqb, qblk], f16)
            v_sb = singles.tile([ktile, nt, d + 1], f16)
            tri_sb = singles.tile([ktile, ktile], f16)

            # Blocks are processed in ASCENDING order; block qb needs k-tile
            # pairs 0..2qb+1, v tiles 0..4qb+3, and its own q block. Small
            # first chunks so compute starts early; per-DMA issue on the
            # sync queue is ~650ns, serial.
            nc.sync.dma_start(out=kp_sb[:, 0:2, :], in_=kp_d[:, 0:2, :])
            nc.sync.dma_start(out=qq_sb[:, 0, :], in_=qq_d[:, 0, :])
            nc.sync.dma_start(out=tri_sb, in_=tri_d)
            nc.sync.dma_start(out=v_sb[:, 0:4, :], in_=v_d[:, 0:4, :])
            nc.sync.dma_start(out=qq_sb[:, 1:4, :], in_=qq_d[:, 1:4, :])
            nc.sync.dma_start(out=kp_sb[:, 2:8, :], in_=kp_d[:, 2:8, :])
            nc.sync.dma_start(out=v_sb[:, 4:16, :], in_=v_d[:, 4:16, :])
            nc.sync.dma_start(out=qq_sb[:, 4:nqb, :], in_=qq_d[:, 4:nqb, :])
            nc.sync.dma_start(out=kp_sb[:, 8:npr, :], in_=kp_d[:, 8:npr, :])
            nc.sync.dma_start(out=v_sb[:, 16:nt, :], in_=v_d[:, 16:nt, :])

            # --- main loop -------------------------------------------------
            def epilogue(acc, qb):
                # normalize: out = acc[0:64] * (1/sums) (sums = row d of acc).
                # reciprocal_approx_fast (1 custom-DVE op, ~18-bit, ~5x faster
                # than InstReciprocal) reads the PSUM row and writes SBUF; a
                # DRAM round-trip broadcasts it across the 64 partitions
                # (partition-step-0 reads are DRAM-only).
                qs = qb * qblk
                # custom-DVE ops misread PSUM operands on HW (measured ~10%
                # error) -- stage the sums row through SBUF first.
                ssum = ob_pool.tile([1, qblk], f32, name="ssum")
                nc.vector.tensor_copy(ssum, acc[d:d + 1, :])
                rsum = ob_pool.tile([1, qblk], f32, name="rsum")
                nc.vector.reciprocal_approx_fast(out=rsum, in_=ssum)
                nc.gpsimd.dma_start(out=rs_d[qb:qb + 1, :], in_=rsum)
                rb = ob_pool.tile([d, qblk], f32, name="rb")
                rs_slice = rs_d[qb:qb + 1, :]
                brd = bass.AP(tensor=rs_slice.tensor,
                              offset=rs_slice.offset,
                              ap=[[0, d], list(rs_slice.ap[-1])])
                nc.gpsimd.dma_start(out=rb, in_=brd)
                ob = ob_pool.tile([d, qblk], f32, name="ob")
                nc.vector.tensor_mul(ob, acc[0:d, :], rb)
                nc.gpsimd.dma_start(out=oT_d[:, qs:qs + qblk], in_=ob)

            # Per k-tile pair: emit the two ROW-TILED MM1s + exp(pair)
            # [+ boundary tri-mask], then the deferred MM2s of the previous
            # pair, so the PE stream interleaves [.. MM1s(p) MM2s(p-1) ..]
            # and fills the exp latency. Diagonal tiles (j = t - tpq*qb >= 0)
            # read/write only their live columns [128*j, qblk) in MM2.
            mm2_q = []   # deferred MM2s: (acc, pb, (t0, t1), qb, tlast)

            def flush_mm2():
                acc_, pb_, tiles_, qb_, tlast_ = mm2_q.pop(0)
                for h, t in enumerate(tiles_):
                    j = t - tpq * qb_
                    cs = ktile * j if j > 0 else 0
                    nc.tensor.matmul(
                        acc_[:, cs:],
                        lhsT=v_sb[:, t, :],
                        rhs=pb_[:, h, cs:],
                        start=(t == 0), stop=(t == tlast_),
                    )
                if tiles_[1] == tlast_:   # last pair: normalize this q-block
                    epilogue(acc_, qb_)

            for qb in range(nqb):
                ntiles = tpq * (qb + 1)
                npairs = ntiles // 2
                tlast = ntiles - 1
                acc = acc_pool.tile([d + 1, qblk], f32, name="acc", tag="acc")
                for p in range(npairs):
                    t0, t1 = 2 * p, 2 * p + 1
                    lg = lg_pool.tile([128, 2, qblk], f32, name="lg")
                    nc.tensor.matmul(
                        lg[:, 0, :],
                        lhsT=kp_sb[0:d, p, :],
                        rhs=qq_sb[0:d, qb, :],
                        start=True, stop=True,
                    )
                    nc.tensor.matmul(
                        lg[:, 1, :],
                        lhsT=kp_sb[d:2 * d, p, :],
                        rhs=qq_sb[d:2 * d, qb, :],
                        start=True, stop=True,
                    )
                    pb = pb_pool.tile([128, 2, qblk], f16, name="pb")
                    # Far-diagonal pair (tiles j=2,3): only columns >= 256
                    # are ever read by MM2 -- exp just that strided slice
                    # (one ACT op, free size 512 instead of 1024).
                    if t0 - tpq * qb == 2:
                        exp_out, exp_in = pb[:, :, 256:], lg[:, :, 256:]
                    else:
                        exp_out, exp_in = pb, lg
                    nc.scalar.activation(
                        exp_out, exp_in, mybir.ActivationFunctionType.Exp,
                        scale=scale)
                    for h, t in ((0, t0), (1, t1)):
                        j = t - tpq * qb
                        if j >= 0:
                            nc.vector.tensor_mul(
                                pb[:, h, ktile * j:ktile * (j + 1)],
                                pb[:, h, ktile * j:ktile * (j + 1)],
                                tri_sb)
                    mm2_q.append((acc, pb, (t0, t1), qb, tlast))
                    if len(mm2_q) > mm2_defer:
                        flush_mm2()
            while mm2_q:
                flush_mm2()

    nc.compile()
    return nc


def _get_nc(key="main", **kw):
    if key not in _NC_CACHE:
        _NC_CACHE[key] = build(**kw)
    return _NC_CACHE[key]


def _prep_core_inputs(q, k, v, attn_mask, b, n=N, d=D, ktile=KTILE,
                      qblk=QBLK):
    nt = n // ktile
    nqb = n // qblk
    npr = nt // 2
    kT = k[b].T.astype(np.float16)    # [d, n]
    qT = q[b].T.astype(np.float16)
    # kpair[0:64, p, :] = kT tile 2p; kpair[64:128, p, :] = kT tile 2p+1
    kpair = np.ascontiguousarray(
        kT.reshape(d, npr, 2, ktile).transpose(2, 0, 1, 3)
    ).reshape(2 * d, npr, ktile)
    # qq: qT blocks duplicated on both partition halves
    qq = np.empty((2 * d, nqb, qblk), dtype=np.float16)
    qq[0:d] = qT.reshape(d, nqb, qblk)
    qq[d:2 * d] = qq[0:d]
    v_aug = np.ones((n, d + 1), dtype=np.float32)
    v_aug[:, :d] = v[b]
    v_aug *= (attn_mask[b] != 0).astype(np.float32)[:, None]
    v_aug = np.ascontiguousarray(
        v_aug.reshape(nt, ktile, d + 1).transpose(1, 0, 2)
    ).astype(np.float16)
    # tri[kk, qc] = 1 iff qc >= kk (keep)
    tri = (np.arange(ktile)[None, :] >= np.arange(ktile)[:, None]
           ).astype(np.float16)
    return {"kpair": kpair, "qq": qq, "v_aug": v_aug, "tri": tri}


def kernel(q, k, v, attn_mask):
    global LAST_RESULTS
    q = np.asarray(q, dtype=np.float32)
    k = np.asarray(k, dtype=np.float32)
    v = np.asarray(v, dtype=np.float32)
    attn_mask = np.asarray(attn_mask)

    from concourse.bass_utils import run_bass_kernel_spmd

    nc = _get_nc()
    in_maps = [_prep_core_inputs(q, k, v, attn_mask, b) for b in range(B)]
    trace = bool(os.environ.get("BASS_TRACE"))
    last_err = None
    for attempt in range(3):
        try:
            LAST_RESULTS = run_bass_kernel_spmd(
                nc, in_maps, core_ids=list(range(B)), trace=trace)
            break
        except Exception as e:  # transient device-unrecoverable states clear
            last_err = e        # on the next execution attempt
            if "UNAVAILABLE" not in str(e) and "unrecoverable" not in str(e):
                raise
            import time as _time

            _time.sleep(2.0)
    else:
        raise last_err

    out = np.empty((B, N, D), dtype=np.float32)
    for b in range(B):
        out[b] = LAST_RESULTS.results[b]["outT"].T
    return out
